# revision 40
# baseline (speedup 1.0000x reference)
"""DeepSeekV2-style MLA prefill attention on 8 Trainium2 NeuronCores.

Tensor-parallel over heads: each core owns 2 of the 16 q heads (q_nope only —
the rope half of q is discarded by the reference), replicates the single
latent kv head, computes its heads' causal attention and a partial o-proj;
the host sums the 8 partial outputs.

All matmuls run in bf16 (fp32 PSUM accumulation) — bf16 streams 1 col/cycle
on the PE and enables fast weight load; fp32/f32r stream at half rate. The
host supplies bf16 inputs so no on-device rounding passes are needed.
Measured end-to-end relative error ~5e-3 vs the fp32 reference.

hs^T and the output are exchanged as pre-tiled arrays ([st2, hid_chunk, 128,
1024] C-contiguous) so every DMA is one contiguous 256KB block — strided
per-partition descriptors (~360ns each) otherwise cap DMA at ~130GB/s.

Layout strategy (matmuls contract over the partition dim):
  - projections computed transposed: q^T/k^T/v^T [d, s] = W^T.T @ hs^T
  - scores^T [l, q] = k^T_chunk.T @ q^T_tile (fp32 PSUM), diagonal tiles
    narrowed to their causal width
  - softmax without max-subtraction (scores are provably small: |s| < ~6)
  - exp on ScalarE (PSUM -> SBUF bf16); only the [128,128] diagonal block
    needs a triangular mask multiply
  - PV: out[q, d+1] += expT_chunk.T @ [v | 1] (bf16); the appended ones
    column accumulates the softmax denominator for free; two qs-pair passes
    so consecutive blocks pipeline through the 4 PSUM accumulator slots
  - normalize by the reciprocal of that column, PE-transpose, partial o-proj
    emitted transposed+tiled bf16; host sums across cores + rebuilds.
"""

import numpy as np
import ml_dtypes
from contextlib import ExitStack

import concourse.bass as bass
import concourse.mybir as mybir
import concourse.tile as tile
from concourse import bacc
from concourse.bass_utils import run_bass_kernel_spmd
from concourse.masks import make_identity

B, S, HID = 2, 2048, 2048
H, D_NOPE, D_ROPE = 16, 128, 64
D = D_NOPE
N_CORES = 8
HPC = H // N_CORES          # heads per core
BS = B * S
SCALE = float(D_NOPE) ** -0.5

F32 = mybir.dt.float32
BF16 = mybir.dt.bfloat16

HC = HID // 128             # 16 hid chunks
ST2 = BS // 1024            # 4 wide s tiles
LCB = S // 128              # 16 l chunks per batch

_cache = {}


def _build():
    if "nc" in _cache:
        return _cache["nc"]

    nc = bacc.Bacc("TRN2", target_bir_lowering=False, debug=False,
                   num_devices=N_CORES)
    # hsT tiled: [st2, hid_chunk, 128, 1024] contiguous blocks
    hsT_d = nc.dram_tensor("hsTt", [ST2, HC, 128, 1024], BF16,
                           kind="ExternalInput").ap()
    wqT_d = nc.dram_tensor("wqT", [HID, HPC * D], BF16, kind="ExternalInput").ap()
    wkT_d = nc.dram_tensor("wkT", [HID, D], BF16, kind="ExternalInput").ap()
    wvT_d = nc.dram_tensor("wvT", [HID, D], BF16, kind="ExternalInput").ap()
    woT_d = nc.dram_tensor("woT", [HPC * D, HID], BF16, kind="ExternalInput").ap()
    # output tiled: [hid_chunk, st2, 128, 1024]
    outT_d = nc.dram_tensor("outTt", [HC, ST2, 128, 1024], BF16,
                            kind="ExternalOutput").ap()

    with ExitStack() as ctx:
        tc = ctx.enter_context(tile.TileContext(nc))
        persist = ctx.enter_context(tc.tile_pool(name="persist", bufs=1))

        wq_sb = persist.tile([128, HC, HPC * D], BF16, tag="wq_sb")
        wk_sb = persist.tile([128, HC, D], BF16, tag="wk_sb")
        wv_sb = persist.tile([128, HC, D], BF16, tag="wv_sb")
        wo_sb = persist.tile([128, HPC, HC, 128], BF16, tag="wo_sb")
        qT_sb = persist.tile([128, HPC, BS], BF16, tag="qT_sb")
        kT_sb = persist.tile([128, BS], BF16, tag="kT_sb")
        vT_sb = persist.tile([128, BS], BF16, tag="vT_sb")
        v_sb = persist.tile([128, B * LCB, D + 1], BF16, tag="v_sb")
        tri_f = persist.tile([128, 128], F32, tag="tri_f")
        tri_sb = persist.tile([128, 128], BF16, tag="tri_sb")
        ident_b = persist.tile([128, 128], BF16, tag="ident_b")
        outT_sb = persist.tile([128, HPC, BS], BF16, tag="outT_sb")

        # ---- constants ----
        wq_src = wqT_d.rearrange("(c p) m -> p c m", p=128)
        wk_src = wkT_d.rearrange("(c p) m -> p c m", p=128)
        wv_src = wvT_d.rearrange("(c p) m -> p c m", p=128)
        make_identity(nc, ident_b[:])
        # tri[x, y] = 1 where x <= y else 0 (diag-block causal mask)
        nc.gpsimd.memset(tri_f[:], 1.0)
        nc.gpsimd.affine_select(
            out=tri_f[:], in_=tri_f[:],
            compare_op=mybir.AluOpType.is_ge,
            fill=0.0, base=0,
            pattern=[[1, 128]], channel_multiplier=-1)
        nc.vector.tensor_copy(tri_sb[:], tri_f[:])
        nc.vector.memset(v_sb[:, :, D:D + 1], 1.0)

        # ---- phase 1: projections q^T (per head), k^T, v^T ----
        # hs tiles are contiguous-block DMAs, deep-prefetched; weight chunks
        # interleave at first use so the first matmuls start ASAP.
        with tc.tile_pool(name="ps_proj", bufs=1, space="PSUM") as ps_proj, \
             tc.tile_pool(name="hs_pool", bufs=10) as hs_pool:
            for st2 in range(ST2):
                # 8 PSUM banks: (2 heads + k + v) x 2 halves
                pq = [[ps_proj.tile([128, 512], F32, tag=f"pq{h}_{hf}",
                                    name=f"pq{h}_{hf}")
                       for hf in range(2)] for h in range(HPC)]
                pk = [ps_proj.tile([128, 512], F32, tag=f"pk_{hf}",
                                   name=f"pk_{hf}") for hf in range(2)]
                pv = [ps_proj.tile([128, 512], F32, tag=f"pv_{hf}",
                                   name=f"pv_{hf}") for hf in range(2)]
                for hcx in range(HC):
                    hst = hs_pool.tile([128, 1024], BF16, tag="hst")
                    nc.sync.dma_start(hst[:], hsT_d[st2, hcx])
                    if st2 == 0:
                        nc.sync.dma_start(wq_sb[:, hcx, :], wq_src[:, hcx, :])
                        nc.sync.dma_start(wk_sb[:, hcx, :], wk_src[:, hcx, :])
                        nc.sync.dma_start(wv_sb[:, hcx, :], wv_src[:, hcx, :])
                        if hcx == HC - 1:
                            nc.sync.dma_start(
                                wo_sb[:],
                                woT_d.rearrange("(h p) (c m) -> p h c m",
                                                p=128, m=128))
                    first, last = hcx == 0, hcx == HC - 1
                    for hf in range(2):
                        hr = hst[:, hf * 512:(hf + 1) * 512]
                        for h in range(HPC):
                            nc.tensor.matmul(
                                pq[h][hf][:],
                                wq_sb[:, hcx, h * D:(h + 1) * D],
                                hr, start=first, stop=last)
                        nc.tensor.matmul(pk[hf][:], wk_sb[:, hcx, :], hr,
                                         start=first, stop=last)
                        nc.tensor.matmul(pv[hf][:], wv_sb[:, hcx, :], hr,
                                         start=first, stop=last)
                for hf in range(2):
                    sl = slice(st2 * 1024 + hf * 512,
                               st2 * 1024 + (hf + 1) * 512)
                    for h in range(HPC):
                        nc.scalar.copy(qT_sb[:, h, sl], pq[h][hf][:])
                    nc.scalar.copy(kT_sb[:, sl], pk[hf][:])
                    nc.vector.tensor_copy(vT_sb[:, sl], pv[hf][:])

        # ---- phases 1b/2/3 share one 8-bank PSUM pool:
        #      tp(1) + ps(2) + outp(4) + po(1) = 8
        main_ps = ctx.enter_context(
            tc.tile_pool(name="main_ps", bufs=1, space="PSUM"))
        att_sb = ctx.enter_context(tc.tile_pool(name="att_sb", bufs=18))
        norm_sb = ctx.enter_context(tc.tile_pool(name="norm_sb", bufs=4))
        stage = ctx.enter_context(tc.tile_pool(name="stage", bufs=3))

        # phase 1b: v^T -> v (natural [l, d] layout) via PE transpose
        for lc in range(B * LCB):
            tp = main_ps.tile([128, 128], BF16, tag="tp", bufs=1, name="tp")
            nc.tensor.transpose(
                tp[:], vT_sb[:, lc * 128:(lc + 1) * 128], ident_b[:])
            nc.vector.tensor_copy(v_sb[:, lc, 0:D], tp[:])

        def emit_oproj(st2, po_tags=("po",)):
            # partial o-proj for s-range [st2*1024, (st2+1)*1024), transposed
            bufmap = {"po": 1, "ps": 2, "outp": 4}
            for hcx in range(HC):
                ob = stage.tile([128, 1024], BF16, tag="ob", name="ob")
                for hf in range(2):
                    sl = slice(st2 * 1024 + hf * 512,
                               st2 * 1024 + (hf + 1) * 512)
                    tag = po_tags[(2 * hcx + hf) % len(po_tags)]
                    po = main_ps.tile([128, 512], F32, tag=tag,
                                      bufs=bufmap[tag], name="po")
                    for h in range(HPC):
                        nc.tensor.matmul(
                            po[:], wo_sb[:, h, hcx, :],
                            outT_sb[:, h, sl],
                            start=(h == 0), stop=(h == HPC - 1))
                    if hf == 0:
                        nc.vector.tensor_copy(
                            ob[:, hf * 512:(hf + 1) * 512], po[:])
                    else:
                        nc.scalar.copy(ob[:, hf * 512:(hf + 1) * 512], po[:])
                nc.sync.dma_start(outT_d[hcx, st2], ob[:])

        def normalize(outp_ap, h, qglob):
            rc = norm_sb.tile([128, 1], F32, tag="rc", name="rc")
            nc.vector.reciprocal(rc[:], outp_ap[:, D:D + 1])
            nm = norm_sb.tile([128, 128], BF16, tag="nm", name="nm")
            nc.vector.tensor_scalar_mul(nm[:], outp_ap[:, 0:D], rc[:])
            tp = main_ps.tile([128, 128], BF16, tag="tp", bufs=1, name="tp2")
            nc.tensor.transpose(tp[:], nm[:], ident_b[:])
            nc.vector.tensor_copy(outT_sb[:, h, qglob:qglob + 128], tp[:])

        # ---- phase 2: causal attention per (batch, q-tile, head),
        #      o-proj woven in as soon as its s-range is complete ----
        for b in range(B):
            qoff = b * S
            for qt in range(S // 512):
                for h in range(HPC):
                    Q = qt * 512
                    nl = Q // 128 + 4
                    exs = []
                    # pass 1: scores + exp for all lc, PV for qs 0,1
                    outp = [main_ps.tile([128, D + 1], F32, tag="outp",
                                         bufs=4, name=f"outp{qs}")
                            for qs in range(2)]
                    for lc in range(nl):
                        m = lc - Q // 128  # >=0: diagonal block index
                        w0 = max(m, 0) * 128  # first useful q col
                        ps = main_ps.tile([128, 512], F32, tag="ps", bufs=2,
                                          name="ps")
                        nc.tensor.matmul(
                            ps[:, w0:512],
                            kT_sb[:, qoff + lc * 128: qoff + (lc + 1) * 128],
                            qT_sb[:, h, qoff + Q + w0: qoff + Q + 512],
                            start=True, stop=True)
                        ex = att_sb.tile([128, 512], BF16, tag="ex",
                                         name="ex")
                        nc.scalar.activation(
                            ex[:, w0:512], ps[:, w0:512],
                            mybir.ActivationFunctionType.Exp, scale=SCALE)
                        if m >= 0:
                            nc.vector.tensor_mul(
                                ex[:, w0:w0 + 128], ex[:, w0:w0 + 128],
                                tri_sb[:])
                        exs.append(ex)
                        for qs in range(2):
                            cq = Q // 128 + qs
                            if lc > cq:
                                continue
                            nc.tensor.matmul(
                                outp[qs][:],
                                ex[:, qs * 128:(qs + 1) * 128],
                                v_sb[:, b * LCB + lc, :],
                                start=(lc == 0), stop=(lc == cq))
                    for qs in range(2):
                        normalize(outp[qs][:], h, qoff + Q + qs * 128)
                    # pass 2: PV for qs 2,3 from the retained exp tiles
                    outp = [main_ps.tile([128, D + 1], F32, tag="outp",
                                         bufs=4, name=f"outp{qs + 2}")
                            for qs in range(2)]
                    for lc in range(nl):
                        for qs in range(2):
                            cq = Q // 128 + qs + 2
                            if lc > cq:
                                continue
                            nc.tensor.matmul(
                                outp[qs][:],
                                exs[lc][:, (qs + 2) * 128:(qs + 3) * 128],
                                v_sb[:, b * LCB + lc, :],
                                start=(lc == 0), stop=(lc == cq))
                    for qs in range(2):
                        normalize(outp[qs][:], h, qoff + Q + (qs + 2) * 128)
                if qt == 1:
                    emit_oproj(2 * b)
                elif qt == 3:
                    # the last o-proj can also use the then-idle psum slots
                    emit_oproj(2 * b + 1,
                               po_tags=("po", "ps", "outp")
                               if b == B - 1 else ("po",))

    nc.compile()
    _cache["nc"] = nc
    return nc


def _bf(x):
    return np.ascontiguousarray(x).astype(ml_dtypes.bfloat16)


def _in_maps(inputs):
    hs = np.asarray(inputs["hidden_states"], np.float32).reshape(BS, HID).T
    hsT = _bf(hs)                                   # [HID, BS]
    # tile into contiguous [st2, hc, 128, 1024] blocks
    hsTt = np.ascontiguousarray(
        hsT.reshape(HC, 128, ST2, 1024).transpose(2, 0, 1, 3))
    Wq = np.asarray(inputs["Wq"], np.float32)
    Wkv = np.asarray(inputs["Wkv"], np.float32)
    Wo = np.asarray(inputs["Wo"], np.float32)

    wkT = _bf(Wkv[:D, :].T)
    wvT = _bf(Wkv[D:2 * D, :].T)
    Wq_r = Wq.reshape(H, D_NOPE + D_ROPE, HID)

    in_maps = []
    for c in range(N_CORES):
        heads = range(c * HPC, (c + 1) * HPC)
        wqT = _bf(np.concatenate([Wq_r[h, :D_NOPE, :] for h in heads], 0).T)
        woT = _bf(np.concatenate(
            [Wo[:, h * D:(h + 1) * D].T for h in heads], 0))
        in_maps.append({
            "hsTt": hsTt, "wqT": wqT, "wkT": wkT, "wvT": wvT, "woT": woT,
        })
    return in_maps


def _gather(results):
    acc = results[0]["outTt"].astype(np.float32)
    for r in results[1:]:
        acc = acc + r["outTt"].astype(np.float32)
    # [hc, st2, 128, 1024] -> outT [HID, BS] -> [B, S, HID]
    outT = acc.transpose(0, 2, 1, 3).reshape(HID, BS)
    return np.ascontiguousarray(outT.T).reshape(B, S, HID)


def run(inputs, trace=False, **kw):
    nc = _build()
    res = run_bass_kernel_spmd(nc, _in_maps(inputs), list(range(N_CORES)),
                               trace=trace, **kw)
    return _gather(res.results), res


def kernel(**inputs):
    out, _ = run(inputs)
    return out


# revision 43
# speedup vs baseline: 1.0031x; 1.0031x over previous
"""DeepSeekV2-style MLA prefill attention on 8 Trainium2 NeuronCores.

Tensor-parallel over heads: each core owns 2 of the 16 q heads (q_nope only —
the rope half of q is discarded by the reference), replicates the single
latent kv head, computes its heads' causal attention and a partial o-proj;
the host sums the 8 partial outputs.

All matmuls run in bf16 (fp32 PSUM accumulation) — bf16 streams 1 col/cycle
on the PE and enables fast weight load; fp32/f32r stream at half rate. The
host supplies bf16 inputs so no on-device rounding passes are needed.
Measured end-to-end relative error ~5e-3 vs the fp32 reference.

hs^T and the output are exchanged as pre-tiled arrays ([st2, hid_chunk, 128,
1024] C-contiguous) so every DMA is one contiguous 256KB block — strided
per-partition descriptors (~360ns each) otherwise cap DMA at ~130GB/s.

Layout strategy (matmuls contract over the partition dim):
  - projections computed transposed: q^T/k^T/v^T [d, s] = W^T.T @ hs^T
  - scores^T [l, q] = k^T_chunk.T @ q^T_tile (fp32 PSUM), diagonal tiles
    narrowed to their causal width
  - softmax without max-subtraction (scores are provably small: |s| < ~6)
  - exp on ScalarE (PSUM -> SBUF bf16); only the [128,128] diagonal block
    needs a triangular mask multiply
  - PV: out[q, d+1] += expT_chunk.T @ [v | 1] (bf16); the appended ones
    column accumulates the softmax denominator for free; two qs-pair passes
    so consecutive blocks pipeline through the 4 PSUM accumulator slots
  - normalize by the reciprocal of that column, PE-transpose, partial o-proj
    emitted transposed+tiled bf16; host sums across cores + rebuilds.
"""

import numpy as np
import ml_dtypes
from contextlib import ExitStack

import concourse.bass as bass
import concourse.mybir as mybir
import concourse.tile as tile
from concourse import bacc
from concourse.bass_utils import run_bass_kernel_spmd
from concourse.masks import make_identity

B, S, HID = 2, 2048, 2048
H, D_NOPE, D_ROPE = 16, 128, 64
D = D_NOPE
N_CORES = 8
HPC = H // N_CORES          # heads per core
BS = B * S
SCALE = float(D_NOPE) ** -0.5

F32 = mybir.dt.float32
BF16 = mybir.dt.bfloat16

HC = HID // 128             # 16 hid chunks
ST2 = BS // 1024            # 4 wide s tiles
LCB = S // 128              # 16 l chunks per batch

_cache = {}


def _build():
    if "nc" in _cache:
        return _cache["nc"]

    nc = bacc.Bacc("TRN2", target_bir_lowering=False, debug=False,
                   num_devices=N_CORES)
    # hsT tiled: [st2, hid_chunk, 128, 1024] contiguous blocks
    hsT_d = nc.dram_tensor("hsTt", [ST2, HC, 128, 1024], BF16,
                           kind="ExternalInput").ap()
    wqT_d = nc.dram_tensor("wqT", [HID, HPC * D], BF16, kind="ExternalInput").ap()
    wkT_d = nc.dram_tensor("wkT", [HID, D], BF16, kind="ExternalInput").ap()
    wvT_d = nc.dram_tensor("wvT", [HID, D], BF16, kind="ExternalInput").ap()
    woT_d = nc.dram_tensor("woT", [HPC * D, HID], BF16, kind="ExternalInput").ap()
    # output tiled: [hid_chunk, st2, 128, 1024]
    outT_d = nc.dram_tensor("outTt", [HC, ST2, 128, 1024], BF16,
                            kind="ExternalOutput").ap()

    with ExitStack() as ctx:
        tc = ctx.enter_context(tile.TileContext(nc))
        persist = ctx.enter_context(tc.tile_pool(name="persist", bufs=1))

        wq_sb = persist.tile([128, HC, HPC * D], BF16, tag="wq_sb")
        wk_sb = persist.tile([128, HC, D], BF16, tag="wk_sb")
        wv_sb = persist.tile([128, HC, D], BF16, tag="wv_sb")
        wo_sb = persist.tile([128, HPC, HC, 128], BF16, tag="wo_sb")
        qT_sb = persist.tile([128, HPC, BS], BF16, tag="qT_sb")
        kT_sb = persist.tile([128, BS], BF16, tag="kT_sb")
        vT_sb = persist.tile([128, BS], BF16, tag="vT_sb")
        v_sb = persist.tile([128, B * LCB, D + 1], BF16, tag="v_sb")
        tri_f = persist.tile([128, 128], F32, tag="tri_f")
        tri_sb = persist.tile([128, 128], BF16, tag="tri_sb")
        ident_b = persist.tile([128, 128], BF16, tag="ident_b")
        outT_sb = persist.tile([128, HPC, BS], BF16, tag="outT_sb")

        # ---- constants ----
        wq_src = wqT_d.rearrange("(c p) m -> p c m", p=128)
        wk_src = wkT_d.rearrange("(c p) m -> p c m", p=128)
        wv_src = wvT_d.rearrange("(c p) m -> p c m", p=128)
        make_identity(nc, ident_b[:])
        # tri[x, y] = 1 where x <= y else 0 (diag-block causal mask)
        nc.gpsimd.memset(tri_f[:], 1.0)
        nc.gpsimd.affine_select(
            out=tri_f[:], in_=tri_f[:],
            compare_op=mybir.AluOpType.is_ge,
            fill=0.0, base=0,
            pattern=[[1, 128]], channel_multiplier=-1)
        nc.vector.tensor_copy(tri_sb[:], tri_f[:])
        nc.vector.memset(v_sb[:, :, D:D + 1], 1.0)

        # ---- phase 1: projections q^T (per head), k^T, v^T ----
        # hs tiles are contiguous-block DMAs, deep-prefetched; weight chunks
        # interleave at first use so the first matmuls start ASAP.
        with tc.tile_pool(name="ps_proj", bufs=1, space="PSUM") as ps_proj, \
             tc.tile_pool(name="hs_pool", bufs=16) as hs_pool:
            for st2 in range(ST2):
                # 8 PSUM banks: (2 heads + k + v) x 2 halves
                pq = [[ps_proj.tile([128, 512], F32, tag=f"pq{h}_{hf}",
                                    name=f"pq{h}_{hf}")
                       for hf in range(2)] for h in range(HPC)]
                pk = [ps_proj.tile([128, 512], F32, tag=f"pk_{hf}",
                                   name=f"pk_{hf}") for hf in range(2)]
                pv = [ps_proj.tile([128, 512], F32, tag=f"pv_{hf}",
                                   name=f"pv_{hf}") for hf in range(2)]
                for hcx in range(HC):
                    hst = hs_pool.tile([128, 1024], BF16, tag="hst")
                    nc.sync.dma_start(hst[:], hsT_d[st2, hcx])
                    if st2 == 0:
                        nc.sync.dma_start(wq_sb[:, hcx, :], wq_src[:, hcx, :])
                        nc.sync.dma_start(wk_sb[:, hcx, :], wk_src[:, hcx, :])
                        nc.sync.dma_start(wv_sb[:, hcx, :], wv_src[:, hcx, :])
                        if hcx == HC - 1:
                            nc.sync.dma_start(
                                wo_sb[:],
                                woT_d.rearrange("(h p) (c m) -> p h c m",
                                                p=128, m=128))
                    first, last = hcx == 0, hcx == HC - 1
                    for hf in range(2):
                        hr = hst[:, hf * 512:(hf + 1) * 512]
                        for h in range(HPC):
                            nc.tensor.matmul(
                                pq[h][hf][:],
                                wq_sb[:, hcx, h * D:(h + 1) * D],
                                hr, start=first, stop=last)
                        nc.tensor.matmul(pk[hf][:], wk_sb[:, hcx, :], hr,
                                         start=first, stop=last)
                        nc.tensor.matmul(pv[hf][:], wv_sb[:, hcx, :], hr,
                                         start=first, stop=last)
                for hf in range(2):
                    sl = slice(st2 * 1024 + hf * 512,
                               st2 * 1024 + (hf + 1) * 512)
                    for h in range(HPC):
                        nc.scalar.copy(qT_sb[:, h, sl], pq[h][hf][:])
                    nc.scalar.copy(kT_sb[:, sl], pk[hf][:])
                    nc.vector.tensor_copy(vT_sb[:, sl], pv[hf][:])

        # ---- phases 1b/2/3 share one 8-bank PSUM pool:
        #      tp(1) + ps(2) + outp(4) + po(1) = 8
        main_ps = ctx.enter_context(
            tc.tile_pool(name="main_ps", bufs=1, space="PSUM"))
        att_sb = ctx.enter_context(tc.tile_pool(name="att_sb", bufs=20))
        norm_sb = ctx.enter_context(tc.tile_pool(name="norm_sb", bufs=4))
        stage = ctx.enter_context(tc.tile_pool(name="stage", bufs=4))

        # phase 1b: v^T -> v (natural [l, d] layout) via PE transpose
        for lc in range(B * LCB):
            tp = main_ps.tile([128, 128], BF16, tag="tp", bufs=1, name="tp")
            nc.tensor.transpose(
                tp[:], vT_sb[:, lc * 128:(lc + 1) * 128], ident_b[:])
            nc.vector.tensor_copy(v_sb[:, lc, 0:D], tp[:])

        def emit_oproj(st2, po_tags=("po",)):
            # partial o-proj for s-range [st2*1024, (st2+1)*1024), transposed
            bufmap = {"po": 1, "ps": 2, "outp": 4}
            for hcx in range(HC):
                ob = stage.tile([128, 1024], BF16, tag="ob", name="ob")
                for hf in range(2):
                    sl = slice(st2 * 1024 + hf * 512,
                               st2 * 1024 + (hf + 1) * 512)
                    tag = po_tags[(2 * hcx + hf) % len(po_tags)]
                    po = main_ps.tile([128, 512], F32, tag=tag,
                                      bufs=bufmap[tag], name="po")
                    for h in range(HPC):
                        nc.tensor.matmul(
                            po[:], wo_sb[:, h, hcx, :],
                            outT_sb[:, h, sl],
                            start=(h == 0), stop=(h == HPC - 1))
                    if hf == 0:
                        nc.vector.tensor_copy(
                            ob[:, hf * 512:(hf + 1) * 512], po[:])
                    else:
                        nc.scalar.copy(ob[:, hf * 512:(hf + 1) * 512], po[:])
                nc.sync.dma_start(outT_d[hcx, st2], ob[:])

        def normalize(outp_ap, h, qglob):
            rc = norm_sb.tile([128, 1], F32, tag="rc", name="rc")
            nc.vector.reciprocal(rc[:], outp_ap[:, D:D + 1])
            nm = norm_sb.tile([128, 128], BF16, tag="nm", name="nm")
            nc.vector.tensor_scalar_mul(nm[:], outp_ap[:, 0:D], rc[:])
            tp = main_ps.tile([128, 128], BF16, tag="tp", bufs=1, name="tp2")
            nc.tensor.transpose(tp[:], nm[:], ident_b[:])
            nc.vector.tensor_copy(outT_sb[:, h, qglob:qglob + 128], tp[:])

        # ---- phase 2: causal attention per (batch, q-tile, head),
        #      o-proj woven in as soon as its s-range is complete ----
        for b in range(B):
            qoff = b * S
            for qt in range(S // 512):
                for h in range(HPC):
                    Q = qt * 512
                    nl = Q // 128 + 4
                    exs = []
                    # pass 1: scores + exp for all lc, PV for qs 0,1
                    outp = [main_ps.tile([128, D + 1], F32, tag="outp",
                                         bufs=4, name=f"outp{qs}")
                            for qs in range(2)]
                    for lc in range(nl):
                        m = lc - Q // 128  # >=0: diagonal block index
                        w0 = max(m, 0) * 128  # first useful q col
                        ps = main_ps.tile([128, 512], F32, tag="ps", bufs=2,
                                          name="ps")
                        nc.tensor.matmul(
                            ps[:, w0:512],
                            kT_sb[:, qoff + lc * 128: qoff + (lc + 1) * 128],
                            qT_sb[:, h, qoff + Q + w0: qoff + Q + 512],
                            start=True, stop=True)
                        ex = att_sb.tile([128, 512], BF16, tag="ex",
                                         name="ex")
                        nc.scalar.activation(
                            ex[:, w0:512], ps[:, w0:512],
                            mybir.ActivationFunctionType.Exp, scale=SCALE)
                        if m >= 0:
                            nc.vector.tensor_mul(
                                ex[:, w0:w0 + 128], ex[:, w0:w0 + 128],
                                tri_sb[:])
                        exs.append(ex)
                        for qs in range(2):
                            cq = Q // 128 + qs
                            if lc > cq:
                                continue
                            nc.tensor.matmul(
                                outp[qs][:],
                                ex[:, qs * 128:(qs + 1) * 128],
                                v_sb[:, b * LCB + lc, :],
                                start=(lc == 0), stop=(lc == cq))
                    for qs in range(2):
                        normalize(outp[qs][:], h, qoff + Q + qs * 128)
                    # pass 2: PV for qs 2,3 from the retained exp tiles
                    outp = [main_ps.tile([128, D + 1], F32, tag="outp",
                                         bufs=4, name=f"outp{qs + 2}")
                            for qs in range(2)]
                    for lc in range(nl):
                        for qs in range(2):
                            cq = Q // 128 + qs + 2
                            if lc > cq:
                                continue
                            nc.tensor.matmul(
                                outp[qs][:],
                                exs[lc][:, (qs + 2) * 128:(qs + 3) * 128],
                                v_sb[:, b * LCB + lc, :],
                                start=(lc == 0), stop=(lc == cq))
                    for qs in range(2):
                        normalize(outp[qs][:], h, qoff + Q + (qs + 2) * 128)
                if qt == 1:
                    emit_oproj(2 * b)
                elif qt == 3:
                    # the last o-proj can also use the then-idle psum slots
                    emit_oproj(2 * b + 1,
                               po_tags=("po", "ps", "outp")
                               if b == B - 1 else ("po",))

    nc.compile()
    _cache["nc"] = nc
    return nc


def _bf(x):
    return np.ascontiguousarray(x).astype(ml_dtypes.bfloat16)


def _in_maps(inputs):
    hs = np.asarray(inputs["hidden_states"], np.float32).reshape(BS, HID).T
    hsT = _bf(hs)                                   # [HID, BS]
    # tile into contiguous [st2, hc, 128, 1024] blocks
    hsTt = np.ascontiguousarray(
        hsT.reshape(HC, 128, ST2, 1024).transpose(2, 0, 1, 3))
    Wq = np.asarray(inputs["Wq"], np.float32)
    Wkv = np.asarray(inputs["Wkv"], np.float32)
    Wo = np.asarray(inputs["Wo"], np.float32)

    wkT = _bf(Wkv[:D, :].T)
    wvT = _bf(Wkv[D:2 * D, :].T)
    Wq_r = Wq.reshape(H, D_NOPE + D_ROPE, HID)

    in_maps = []
    for c in range(N_CORES):
        heads = range(c * HPC, (c + 1) * HPC)
        wqT = _bf(np.concatenate([Wq_r[h, :D_NOPE, :] for h in heads], 0).T)
        woT = _bf(np.concatenate(
            [Wo[:, h * D:(h + 1) * D].T for h in heads], 0))
        in_maps.append({
            "hsTt": hsTt, "wqT": wqT, "wkT": wkT, "wvT": wvT, "woT": woT,
        })
    return in_maps


def _gather(results):
    acc = results[0]["outTt"].astype(np.float32)
    for r in results[1:]:
        acc = acc + r["outTt"].astype(np.float32)
    # [hc, st2, 128, 1024] -> outT [HID, BS] -> [B, S, HID]
    outT = acc.transpose(0, 2, 1, 3).reshape(HID, BS)
    return np.ascontiguousarray(outT.T).reshape(B, S, HID)


def run(inputs, trace=False, **kw):
    nc = _build()
    res = run_bass_kernel_spmd(nc, _in_maps(inputs), list(range(N_CORES)),
                               trace=trace, **kw)
    return _gather(res.results), res


def kernel(**inputs):
    out, _ = run(inputs)
    return out
